# revision 1
# baseline (speedup 1.0000x reference)
"""AdaConv Trainium2 kernel — 8-core data parallel (1 sample per core).

Math (per sample):
  dw    = conv2d(style[512,4,4], dw_w[1024,512,2,2], VALID) + dw_b   -> [1024, 3, 3]
  s     = mean(style, spatial)                                        -> [512]
  pw_kn = s @ pw_kn_w.T + pw_kn_b  -> [64, 4, 4] per-group 1x1 matrices
  pw_b  = s @ pw_b_w.T + pw_b_b    -> [256]
  xn    = instance_norm(x) ; xpad = reflect_pad(xn, 1)
  y     = grouped_conv3x3(xpad, dw, groups=64); out = grouped1x1(y, pw_kn) + pw_b

Device strategy:
  - Fold the grouped 1x1 into the 3x3:  Weff[g,o,i,t] = sum_m pw_kn[g,o,m]*dw[4g+m,i,t].
    Conv becomes one grouped 3x3 with weights Weff, bias pw_b.
  - Grouped conv -> per-tap block-diagonal 128x128 matmuls (bf16), PSUM-accumulated
    over the 9 taps, 2 channel halves.
  - Instance norm applied as per-channel affine (scale=rsqrt(var+eps), bias=-mean*scale)
    fused into an in-place ScalarE pass over the padded bf16 buffer.
  - Block-diagonal weight assembly via a DRAM bounce: diagonal-strided DMA scatter
    into a zeroed DRAM tile, then a dense load back; fold = one 128x128 matmul
    per (tap, half):  Weff_bd = D_bd.T @ P_bd.
"""

import sys

sys.path.insert(0, "/opt/trn_rl_repo")

import numpy as np
import ml_dtypes

BF16 = ml_dtypes.bfloat16

N_CORES = 8
C = 256
SD = 512
H = W = 128
HW = H * W
WP = 130  # padded height/width
NT = 9  # taps

_PROG = None


def _build_program():
    import concourse.bass as bass
    import concourse.mybir as mybir
    from concourse.tile import TileContext

    f32 = mybir.dt.float32
    bf16 = mybir.dt.bfloat16
    AF = mybir.ActivationFunctionType
    AX = mybir.AxisListType
    ALU = mybir.AluOpType

    nc = bass.Bass()

    x_p = nc.declare_dram_parameter("x", [C, HW], f32, isOutput=False)
    sty_p = nc.declare_dram_parameter("style", [SD, 16], f32, isOutput=False)
    sc_p = nc.declare_dram_parameter("stycol", [2048, NT], bf16, isOutput=False)
    dww_p = nc.declare_dram_parameter("dww", [2048, 1024], bf16, isOutput=False)
    dwb_p = nc.declare_dram_parameter("dwb", [1, 1024], bf16, isOutput=False)
    knw_p = nc.declare_dram_parameter("knw", [SD, 1024], bf16, isOutput=False)
    knb_p = nc.declare_dram_parameter("knb", [1, 1024], bf16, isOutput=False)
    pbw_p = nc.declare_dram_parameter("pbw", [SD, C], bf16, isOutput=False)
    pbb_p = nc.declare_dram_parameter("pbb", [1, C], bf16, isOutput=False)
    mask_p = nc.declare_dram_parameter("mask", [128, 128], bf16, isOutput=False)
    out_p = nc.declare_dram_parameter("out", [C, HW], bf16, isOutput=True)

    from contextlib import ExitStack

    with TileContext(nc) as tc, ExitStack() as ctx:
        spool = ctx.enter_context(tc.tile_pool(name="spool", bufs=1))
        stage = ctx.enter_context(tc.tile_pool(name="stage", bufs=4))
        xpadp = ctx.enter_context(tc.tile_pool(name="xpadp", bufs=1))
        ypool = ctx.enter_context(tc.tile_pool(name="ypool", bufs=4))
        wq = ctx.enter_context(tc.tile_pool(name="wq", bufs=1))
        wpool = ctx.enter_context(tc.tile_pool(name="wpool", bufs=1))
        genps_ctx = ExitStack()
        gps = genps_ctx.enter_context(tc.tile_pool(name="gps", bufs=1, space="PSUM"))

        # ======== small constants ========
        ones_sb = spool.tile([1, 16], bf16, tag="ones")
        nc.vector.memset(ones_sb[:], 1.0)
        eps_sb = spool.tile([128, 1], f32, tag="eps")
        nc.vector.memset(eps_sb[:], 1e-5)

        # ======== weight loads ========
        # dww first on the scalar queue (DIRECT2D transfers occupy the issuing
        # engine, so queue order = availability order); knw/pbw ride the sync
        # queue after the x h0 stream.
        stycol_sb = spool.tile([128, 16, NT], bf16, tag="stycol")
        nc.scalar.dma_start(
            out=stycol_sb[:], in_=bass.AP(sc_p, 0, [[NT, 128], [128 * NT, 16], [1, NT]])
        )
        style_sb = spool.tile([128, 4, 16], f32, tag="style")
        nc.scalar.dma_start(
            out=style_sb[:], in_=bass.AP(sty_p, 0, [[16, 128], [128 * 16, 4], [1, 16]])
        )
        dwb_sb = spool.tile([1, 1024], bf16, tag="dwb")
        nc.scalar.dma_start(out=dwb_sb[:], in_=dwb_p[:, :])
        knb_sb = spool.tile([1, 1024], bf16, tag="knb")
        nc.scalar.dma_start(out=knb_sb[:], in_=knb_p[:, :])
        pbb_sb = spool.tile([1, C], bf16, tag="pbb")
        nc.scalar.dma_start(out=pbb_sb[:], in_=pbb_p[:, :])
        mask_sb = wpool.tile([128, 128], bf16, tag="mask")
        nc.scalar.dma_start(
            out=mask_sb[:], in_=bass.AP(mask_p, 0, [[128, 128], [1, 128]])
        )
        dww_sb = wpool.tile([128, 16, 1024], bf16, tag="dww")
        for kc in range(16):
            nc.scalar.dma_start(
                out=dww_sb[:, kc, :],
                in_=bass.AP(dww_p, kc * 128 * 1024, [[1024, 128], [1, 1024]]),
            )

        # ======== s = mean(style) ========
        s_f32 = spool.tile([128, 4], f32, tag="sf32")
        nc.vector.tensor_reduce(out=s_f32[:], in_=style_sb[:], axis=AX.X, op=ALU.add)
        s_bf = spool.tile([128, 4], bf16, tag="sbf")
        nc.scalar.activation(out=s_bf[:], in_=s_f32[:], func=AF.Copy, scale=1.0 / 16.0)

        # ======== generate dw -> Dsrc[(gg,m), (h,i,t)] ========
        dw_ps = gps.tile([128, 2, 4, NT], f32, tag="dwps")
        for h in range(2):
            for i in range(4):
                c0 = 512 * h + 128 * i
                for kc in range(16):
                    nc.tensor.matmul(
                        out=dw_ps[:, h, i, :],
                        lhsT=dww_sb[:, kc, c0 : c0 + 128],
                        rhs=stycol_sb[:, kc, :],
                        start=(kc == 0),
                        stop=False,
                    )
                nc.tensor.matmul(
                    out=dw_ps[:, h, i, :],
                    lhsT=dwb_sb[:1, c0 : c0 + 128],
                    rhs=ones_sb[:1, :NT],
                    start=False,
                    stop=True,
                )
        dsrc_sb = spool.tile([128, 2, 4, NT], bf16, tag="dsrc")
        nc.scalar.activation(
            out=dsrc_sb.rearrange("p a b c -> p (a b c)"),
            in_=dw_ps.rearrange("p a b c -> p (a b c)"),
            func=AF.Copy,
        )

        # ======== x h0 stream + knw/pbw loads on sync ========
        xpads, a_ts, mean_bfs, mvs = [], [], [], []

        def stream_half(h, cast_engine):
            xpad = xpadp.tile([128, WP, WP], bf16, tag=f"xpad{h}", name=f"xpad{h}")
            xpads.append(xpad)
            sstats = spool.tile([128, 32, 6], f32, tag=f"sstats{h}", name=f"ss{h}")
            for j in range(8):
                st = stage.tile([128, 2048], f32, tag="stage", name="st")
                nc.sync.dma_start(
                    out=st[:],
                    in_=bass.AP(x_p, h * 128 * HW + j * 2048, [[HW, 128], [1, 2048]]),
                )
                for q in range(4):
                    nc.vector.bn_stats(
                        out=sstats[:, j * 4 + q, :], in_=st[:, q * 512 : (q + 1) * 512]
                    )
                stv = st.rearrange("p (r x) -> p r x", x=128)
                cast_engine.tensor_copy(
                    out=xpad[:, 1 + 16 * j : 1 + 16 * (j + 1), 1:129], in_=stv
                )
                ra, rb = 1 + 16 * j, 1 + 16 * (j + 1)
                cast_engine.tensor_copy(
                    out=xpad[:, ra:rb, 0:1], in_=xpad[:, ra:rb, 2:3]
                )
                cast_engine.tensor_copy(
                    out=xpad[:, ra:rb, 129:130], in_=xpad[:, ra:rb, 127:128]
                )
            cast_engine.tensor_copy(out=xpad[:, 0, :], in_=xpad[:, 2, :])
            cast_engine.tensor_copy(out=xpad[:, 129, :], in_=xpad[:, 127, :])
            mv = spool.tile([128, 2], f32, tag=f"mv{h}", name=f"mv{h}")
            nc.vector.bn_aggr(out=mv[:], in_=sstats[:])
            mvs.append(mv)

        stream_half(0, nc.gpsimd)
        knw_sb = wpool.tile([128, 4, 1024], bf16, tag="knw")
        nc.sync.dma_start(
            out=knw_sb[:],
            in_=bass.AP(knw_p, 0, [[1024, 128], [128 * 1024, 4], [1, 1024]]),
        )
        pbw_sb = wpool.tile([128, 4, C], bf16, tag="pbw")
        nc.sync.dma_start(
            out=pbw_sb[:], in_=bass.AP(pbw_p, 0, [[C, 128], [128 * C, 4], [1, C]])
        )

        # ======== generate pw_kn -> Psrc[(gg,m), (h,o)] ========
        kn_ps = gps.tile([128, 2, 4], f32, tag="knps")
        for h in range(2):
            for o in range(4):
                c0 = 512 * h + 128 * o
                for kc in range(4):
                    nc.tensor.matmul(
                        out=kn_ps[:, h, o : o + 1],
                        lhsT=knw_sb[:, kc, c0 : c0 + 128],
                        rhs=s_bf[:, kc : kc + 1],
                        start=(kc == 0),
                        stop=False,
                    )
                nc.tensor.matmul(
                    out=kn_ps[:, h, o : o + 1],
                    lhsT=knb_sb[:1, c0 : c0 + 128],
                    rhs=ones_sb[:1, :1],
                    start=False,
                    stop=True,
                )
        psrc_sb = spool.tile([128, 2, 4], bf16, tag="psrc")
        nc.scalar.activation(
            out=psrc_sb.rearrange("p a b -> p (a b)"),
            in_=kn_ps.rearrange("p a b -> p (a b)"),
            func=AF.Copy,
        )

        # ======== pw_bias -> [128, 2] f32 ========
        pwb_ps = gps.tile([128, 2], f32, tag="pwbps")
        for h in range(2):
            for kc in range(4):
                nc.tensor.matmul(
                    out=pwb_ps[:, h : h + 1],
                    lhsT=pbw_sb[:, kc, 128 * h : 128 * h + 128],
                    rhs=s_bf[:, kc : kc + 1],
                    start=(kc == 0),
                    stop=False,
                )
            nc.tensor.matmul(
                out=pwb_ps[:, h : h + 1],
                lhsT=pbb_sb[:1, 128 * h : 128 * h + 128],
                rhs=ones_sb[:1, :1],
                start=False,
                stop=True,
            )
        pwbias_sb = spool.tile([128, 2], f32, tag="pwbias")
        nc.scalar.activation(out=pwbias_sb[:], in_=pwb_ps[:], func=AF.Copy)
        genps_ctx.close()
        cps = ctx.enter_context(tc.tile_pool(name="cps", bufs=2, space="PSUM"))

        # ======== per-half: stats->a, block-diag, fold, bias2, conv ========
        wconv_sb = wq.tile([128, NT * 2, 128], bf16, tag="wconv")
        pwbias2_sb = spool.tile([128, 2], f32, tag="pwbias2")
        dbd_sb = wpool.tile([128, NT * 2, 128], bf16, tag="dbd_sb")
        pbd_sb = wpool.tile([128, 2, 128], bf16, tag="pbd_sb")

        def fold_half(h):
            mv = mvs[h]
            a_t = spool.tile([128, 1], f32, tag=f"a{h}", name=f"a{h}")
            nc.scalar.activation(
                out=a_t[:], in_=mv[:, 1:2], func=AF.Sqrt, bias=eps_sb[:]
            )
            nc.vector.reciprocal(out=a_t[:], in_=a_t[:])
            mean_bf = spool.tile([128, 1], bf16, tag=f"mb{h}", name=f"mb{h}")
            nc.vector.tensor_copy(out=mean_bf[:], in_=mv[:, 0:1])
            # block-diag P, D (mask * broadcast), D rows scaled by a
            sl = psrc_sb[:, h, :]
            rep = bass.AP(sl.tensor, sl.offset, [sl.ap[0], [0, 32], [1, 4]])
            nc.vector.tensor_mul(pbd_sb[:, h, :], mask_sb[:], rep)
            for t in range(NT):
                sl = dsrc_sb[:, h, :, t]
                rep = bass.AP(sl.tensor, sl.offset, [sl.ap[0], [0, 32], [NT, 4]])
                nc.vector.tensor_mul(dbd_sb[:, t * 2 + h, :], mask_sb[:], rep)
                nc.vector.tensor_scalar_mul(
                    dbd_sb[:, t * 2 + h, :], dbd_sb[:, t * 2 + h, :], a_t[:]
                )
            for t in range(NT):
                f_ps = cps.tile([128, 1024], f32, tag="conv0", name="f_ps")
                nc.tensor.matmul(
                    out=f_ps[:, :128],
                    lhsT=dbd_sb[:, t * 2 + h, :],
                    rhs=pbd_sb[:, h, :],
                    start=True,
                    stop=True,
                )
                nc.scalar.activation(
                    out=wconv_sb[:, t * 2 + h, :], in_=f_ps[:, :128], func=AF.Copy
                )
            b_ps = cps.tile([128, 1024], f32, tag="conv1", name="b_ps")
            for t in range(NT):
                nc.tensor.matmul(
                    out=b_ps[:, 0:1],
                    lhsT=wconv_sb[:, t * 2 + h, :],
                    rhs=mean_bf[:],
                    start=(t == 0),
                    stop=(t == NT - 1),
                )
            nc.vector.tensor_sub(
                pwbias2_sb[:, h : h + 1], pwbias_sb[:, h : h + 1], b_ps[:, 0:1]
            )

        def conv_half(h):
            xpad = xpads[h]
            for jb in range(8):
                pss = [
                    cps.tile([128, 1024], f32, tag=f"conv{ci}", name=f"c_ps{ci}")
                    for ci in range(2)
                ]
                for t in range(NT):
                    dy, dx = t // 3, t % 3
                    for ci in range(2):
                        j = 2 * jb + ci
                        for hk in range(2):
                            rhs = xpad[
                                :,
                                8 * j + 4 * hk + dy : 8 * j + 4 * hk + dy + 4,
                                dx : dx + 128,
                            ]
                            nc.tensor.matmul(
                                out=pss[ci][:, 512 * hk : 512 * hk + 512],
                                lhsT=wconv_sb[:, t * 2 + h, :],
                                rhs=rhs,
                                start=(t == 0),
                                stop=(t == NT - 1),
                            )
                for ci in range(2):
                    j = 2 * jb + ci
                    yt = ypool.tile([128, 1024], bf16, tag="yt", name="yt")
                    nc.scalar.activation(
                        out=yt[:],
                        in_=pss[ci][:],
                        func=AF.Identity,
                        bias=pwbias2_sb[:, h : h + 1],
                        scale=1.0,
                    )
                    nc.gpsimd.dma_start(
                        out=bass.AP(
                            out_p, h * 128 * HW + j * 1024, [[HW, 128], [1, 1024]]
                        ),
                        in_=yt[:],
                    )

        fold_half(0)
        stream_half(1, nc.vector)
        conv_half(0)
        fold_half(1)
        conv_half(1)

    _enforce_wait_budget(nc, mybir)
    return nc


def _enforce_wait_budget(nc, mybir):
    """Hoist excess sync waits into standalone EventSemaphore instructions.

    HW instruction EVENTS hold 1 wait (+1 update); EventSemaphore holds 2.
    Tile occasionally attaches more, which walrus rejects ("Too many sync
    wait commands").  Inserting the extra waits as EventSemaphore ops just
    before the instruction in the same engine stream is semantically
    identical (engine blocks until the semaphore condition holds).
    """
    n_hoisted = 0
    for bname, bassbb in list(nc.bb_map.items()):
        inner = getattr(bassbb, "bb", bassbb)
        insts = inner.instructions
        new_list = []
        for inst in insts:
            cls = type(inst).__name__
            cap = 2 if cls == "InstEventSemaphore" else 1
            si = inst.sync_info
            if si is None:
                new_list.append(inst)
                continue
            waits = list(si.on_wait or [])
            if len(waits) > cap:
                for w in waits[:-cap]:
                    n_hoisted += 1
                    ev = mybir.InstEventSemaphore(
                        name=f"xw-{n_hoisted}-{inst.name}",
                        engine=inst.engine,
                        ins=[],
                        outs=[],
                        sync_info=mybir.SyncInfo(on_wait=[w], on_update=[]),
                    )
                    new_list.append(ev)
                si.on_wait = waits[-cap:]
                inst.sync_info = si
            new_list.append(inst)
        insts[:] = new_list


def _host_prep(style_encoding, predicted, dw_w, dw_b, pw_kn_w, pw_kn_b, pw_b_w, pw_b_b):
    # Device generates dw/pw_kn with channels on PSUM partitions in (gg, m)
    # order, chunked by (h, i) resp. (h, o).  Permute weight output columns
    # accordingly:
    #   perm_d[512h + 128i + 4gg + m] = 16*(32h+gg) + 4m + i
    #   perm_k[512h + 128o + 4gg + m] = 16*(32h+gg) + 4o + m
    hh, ii, gg, mm = np.meshgrid(
        np.arange(2), np.arange(4), np.arange(32), np.arange(4), indexing="ij"
    )
    perm_d = (16 * (32 * hh + gg) + 4 * mm + ii).reshape(-1)
    perm_k = (16 * (32 * hh + gg) + 4 * ii + mm).reshape(-1)  # ii plays 'o' here
    dww_t = np.ascontiguousarray(dw_w.reshape(1024, 2048)[perm_d].T).astype(BF16)
    knw_t = np.ascontiguousarray(pw_kn_w.reshape(1024, SD)[perm_k].T).astype(BF16)
    pbw_t = np.ascontiguousarray(pw_b_w.reshape(C, SD).T).astype(BF16)
    dwb = dw_b[perm_d].reshape(1, 1024).astype(BF16)
    knb = pw_kn_b[perm_k].reshape(1, 1024).astype(BF16)
    pbb = pw_b_b.reshape(1, C).astype(BF16)
    mask = np.kron(np.eye(32, dtype=np.float32), np.ones((4, 4), np.float32)).astype(
        BF16
    )
    in_maps = []
    for n in range(N_CORES):
        s = style_encoding[n]
        cols = np.empty((SD, 2, 2, 3, 3), np.float32)
        for ky in range(2):
            for kx in range(2):
                cols[:, ky, kx] = s[:, ky : ky + 3, kx : kx + 3]
        stycol = np.ascontiguousarray(cols.reshape(2048, NT)).astype(BF16)
        in_maps.append(
            dict(
                x=np.ascontiguousarray(predicted[n].reshape(C, HW)),
                style=np.ascontiguousarray(s.reshape(SD, 16)),
                stycol=stycol,
                dww=dww_t,
                dwb=dwb,
                knw=knw_t,
                knb=knb,
                pbw=pbw_t,
                pbb=pbb,
                mask=mask,
            )
        )
    return in_maps


def get_program():
    global _PROG
    if _PROG is None:
        _PROG = _build_program()
    return _PROG


def run(in_maps, **kw):
    from concourse.bass_utils import run_bass_kernel_spmd

    nc = get_program()
    return run_bass_kernel_spmd(nc, in_maps, core_ids=list(range(N_CORES)), **kw)


def kernel(**inputs):
    in_maps = _host_prep(**{k: np.asarray(v) for k, v in inputs.items()})
    res = run(in_maps)
    out = np.stack(
        [np.asarray(res.results[n]["out"]).reshape(C, H, W) for n in range(N_CORES)]
    )
    return out.astype(np.float32)



# revision 8
# speedup vs baseline: 1.1743x; 1.1743x over previous
"""AdaConv Trainium2 kernel — 8-core data parallel (1 sample per core).

Math (per sample):
  dw    = conv2d(style[512,4,4], dw_w[1024,512,2,2], VALID) + dw_b   -> [1024, 3, 3]
  s     = mean(style, spatial)                                        -> [512]
  pw_kn = s @ pw_kn_w.T + pw_kn_b  -> [64, 4, 4] per-group 1x1 matrices
  pw_b  = s @ pw_b_w.T + pw_b_b    -> [256]
  xn    = instance_norm(x) ; xpad = reflect_pad(xn, 1)
  y     = grouped_conv3x3(xpad, dw, groups=64); out = grouped1x1(y, pw_kn) + pw_b

Device strategy:
  - Fold the grouped 1x1 into the 3x3:  Weff[g,o,i,t] = sum_m pw_kn[g,o,m]*dw[4g+m,i,t].
    Conv becomes one grouped 3x3 with weights Weff, bias pw_b.
  - Grouped conv -> per-tap block-diagonal 128x128 matmuls (bf16), PSUM-accumulated
    over the 9 taps, 2 channel halves.
  - Instance norm folded into weights: the rsqrt scale a[i] multiplies the
    in-channel partitions of the folded weight (applied on the f_ps PSUM
    evacuation, whose partition dim IS the in-channel (gg,i) layout), and the
    mean correction enters the conv bias via a wconv @ mean matmul.
  - Pipeline: h0 weights stream on the gpsimd DMA queue concurrently with the
    x h0 stream on sync; all of the fold except the final a-scaled evac runs
    before stats complete; h1 stream/gen/fold interleave into conv h0.
"""

import sys

sys.path.insert(0, "/opt/trn_rl_repo")

import numpy as np
import ml_dtypes

BF16 = ml_dtypes.bfloat16

N_CORES = 8
C = 256
SD = 512
H = W = 128
HW = H * W
WP = 130  # padded height/width
NT = 9  # taps

_PROG = None


def _build_program():
    import concourse.bass as bass
    import concourse.mybir as mybir
    from concourse.tile import TileContext

    f32 = mybir.dt.float32
    bf16 = mybir.dt.bfloat16
    AF = mybir.ActivationFunctionType
    AX = mybir.AxisListType
    ALU = mybir.AluOpType

    nc = bass.Bass()

    x_p = nc.declare_dram_parameter("x", [C, HW], f32, isOutput=False)
    sty_p = nc.declare_dram_parameter("style", [SD, 16], f32, isOutput=False)
    sc_p = nc.declare_dram_parameter("stycol", [2048, NT], bf16, isOutput=False)
    dww_p = nc.declare_dram_parameter("dww", [2048, 1024], bf16, isOutput=False)
    dwb_p = nc.declare_dram_parameter("dwb", [1, 1024], bf16, isOutput=False)
    knw_p = nc.declare_dram_parameter("knw", [SD, 1024], bf16, isOutput=False)
    knb_p = nc.declare_dram_parameter("knb", [1, 1024], bf16, isOutput=False)
    pbw_p = nc.declare_dram_parameter("pbw", [SD, C], bf16, isOutput=False)
    pbb_p = nc.declare_dram_parameter("pbb", [1, C], bf16, isOutput=False)
    mask_p = nc.declare_dram_parameter("mask", [128, 128], bf16, isOutput=False)
    out_p = nc.declare_dram_parameter("out", [C, HW], bf16, isOutput=True)

    from contextlib import ExitStack

    with TileContext(nc) as tc, ExitStack() as ctx:
        spool = ctx.enter_context(tc.tile_pool(name="spool", bufs=1))
        stage = ctx.enter_context(tc.tile_pool(name="stage", bufs=4))
        xpadp = ctx.enter_context(tc.tile_pool(name="xpadp", bufs=1))
        ypool = ctx.enter_context(tc.tile_pool(name="ypool", bufs=4))
        wq = ctx.enter_context(tc.tile_pool(name="wq", bufs=1))
        wpool = ctx.enter_context(tc.tile_pool(name="wpool", bufs=1))
        gps = ctx.enter_context(tc.tile_pool(name="gps", bufs=1, space="PSUM"))
        fpool = ctx.enter_context(tc.tile_pool(name="fpool", bufs=1, space="PSUM"))
        cps = ctx.enter_context(tc.tile_pool(name="cps", bufs=2, space="PSUM"))

        # ======== small constants ========
        ones_sb = spool.tile([1, 16], bf16, tag="ones")
        nc.vector.memset(ones_sb[:], 1.0)
        eps_sb = spool.tile([128, 1], f32, tag="eps")
        nc.vector.memset(eps_sb[:], 1e-5)

        # ======== small DMAs (scalar queue) ========
        stycol_sb = spool.tile([128, 16, NT], bf16, tag="stycol")
        nc.scalar.dma_start(
            out=stycol_sb[:], in_=bass.AP(sc_p, 0, [[NT, 128], [128 * NT, 16], [1, NT]])
        )
        style_sb = spool.tile([128, 4, 16], f32, tag="style")
        nc.scalar.dma_start(
            out=style_sb[:], in_=bass.AP(sty_p, 0, [[16, 128], [128 * 16, 4], [1, 16]])
        )
        dwb_sb = spool.tile([1, 1024], bf16, tag="dwb")
        nc.scalar.dma_start(out=dwb_sb[:], in_=dwb_p[:, :])
        knb_sb = spool.tile([1, 1024], bf16, tag="knb")
        nc.scalar.dma_start(out=knb_sb[:], in_=knb_p[:, :])
        pbb_sb = spool.tile([1, C], bf16, tag="pbb")
        nc.scalar.dma_start(out=pbb_sb[:], in_=pbb_p[:, :])
        mask_sb = wpool.tile([128, 128], bf16, tag="mask")
        nc.scalar.dma_start(
            out=mask_sb[:], in_=bass.AP(mask_p, 0, [[128, 128], [1, 128]])
        )

        # ======== s = mean(style) ========
        s_f32 = spool.tile([128, 4], f32, tag="sf32")
        nc.vector.tensor_reduce(out=s_f32[:], in_=style_sb[:], axis=AX.X, op=ALU.add)
        s_bf = spool.tile([128, 4], bf16, tag="sbf")
        nc.scalar.activation(out=s_bf[:], in_=s_f32[:], func=AF.Copy, scale=1.0 / 16.0)

        # ======== per-half weight DMAs ========
        dww_sb = wpool.tile([128, 16, 1024], bf16, tag="dww")
        knw_sb = wpool.tile([128, 4, 1024], bf16, tag="knw")
        pbw_sb = wpool.tile([128, 4, C], bf16, tag="pbw")

        def load_weights_half(h, eng):
            for kc in range(16):
                eng.dma_start(
                    out=dww_sb[:, kc, 512 * h : 512 * h + 512],
                    in_=bass.AP(
                        dww_p, kc * 128 * 1024 + 512 * h, [[1024, 128], [1, 512]]
                    ),
                )
            eng.dma_start(
                out=knw_sb[:, :, 512 * h : 512 * h + 512],
                in_=bass.AP(
                    knw_p, 512 * h, [[1024, 128], [128 * 1024, 4], [1, 512]]
                ),
            )
            eng.dma_start(
                out=pbw_sb[:, :, 128 * h : 128 * h + 128],
                in_=bass.AP(pbw_p, 128 * h, [[C, 128], [128 * C, 4], [1, 128]]),
            )

        load_weights_half(0, nc.gpsimd)

        # ======== generators (per half) ========
        # One PSUM bank carved by hand: dw [0:72], kn [72:80], pwb [80:82],
        # b [82:84]  (f32 elements per partition).
        gall = gps.tile([128, 128], f32, tag="gall")
        dsrc_sb = spool.tile([128, 2, 4, NT], bf16, tag="dsrc")
        psrc_sb = spool.tile([128, 2, 4], bf16, tag="psrc")
        pwbias_sb = spool.tile([128, 2], f32, tag="pwbias")

        def dw_ps_v(h, i):
            o = h * 36 + i * NT
            return gall[:, o : o + NT]

        def kn_ps_v(h, o):
            q = 72 + 4 * h + o
            return gall[:, q : q + 1]

        def pwb_ps_v(h):
            return gall[:, 80 + h : 81 + h]

        def b_ps_v(h):
            return gall[:, 82 + h : 83 + h]

        def dwgen_half(h):
            for i in range(4):
                c0 = 512 * h + 128 * i
                for kc in range(16):
                    nc.tensor.matmul(
                        out=dw_ps_v(h, i),
                        lhsT=dww_sb[:, kc, c0 : c0 + 128],
                        rhs=stycol_sb[:, kc, :],
                        start=(kc == 0),
                        stop=False,
                    )
                nc.tensor.matmul(
                    out=dw_ps_v(h, i),
                    lhsT=dwb_sb[:1, c0 : c0 + 128],
                    rhs=ones_sb[:1, :NT],
                    start=False,
                    stop=True,
                )
            nc.scalar.activation(
                out=dsrc_sb[:, h].rearrange("p a b -> p (a b)"),
                in_=gall[:, h * 36 : h * 36 + 36],
                func=AF.Copy,
            )

        def kngen_half(h):
            for o in range(4):
                c0 = 512 * h + 128 * o
                for kc in range(4):
                    nc.tensor.matmul(
                        out=kn_ps_v(h, o),
                        lhsT=knw_sb[:, kc, c0 : c0 + 128],
                        rhs=s_bf[:, kc : kc + 1],
                        start=(kc == 0),
                        stop=False,
                    )
                nc.tensor.matmul(
                    out=kn_ps_v(h, o),
                    lhsT=knb_sb[:1, c0 : c0 + 128],
                    rhs=ones_sb[:1, :1],
                    start=False,
                    stop=True,
                )
            nc.scalar.activation(
                out=psrc_sb[:, h], in_=gall[:, 72 + 4 * h : 72 + 4 * h + 4], func=AF.Copy
            )
            for kc in range(4):
                nc.tensor.matmul(
                    out=pwb_ps_v(h),
                    lhsT=pbw_sb[:, kc, 128 * h : 128 * h + 128],
                    rhs=s_bf[:, kc : kc + 1],
                    start=(kc == 0),
                    stop=False,
                )
            nc.tensor.matmul(
                out=pwb_ps_v(h),
                lhsT=pbb_sb[:1, 128 * h : 128 * h + 128],
                rhs=ones_sb[:1, :1],
                start=False,
                stop=True,
            )
            nc.vector.tensor_copy(out=pwbias_sb[:, h : h + 1], in_=pwb_ps_v(h))

        # ======== x streaming (per half, chunked) ========
        xpads = [
            xpadp.tile([128, WP, WP], bf16, tag="xpad0", name="xpad0"),
            xpadp.tile([128, WP, WP], bf16, tag="xpad1", name="xpad1"),
        ]
        sstats = [
            spool.tile([128, 32, 6], f32, tag="ss0", name="ss0"),
            spool.tile([128, 32, 6], f32, tag="ss1", name="ss1"),
        ]
        mvs = [
            spool.tile([128, 2], f32, tag="mv0", name="mv0"),
            spool.tile([128, 2], f32, tag="mv1", name="mv1"),
        ]

        def stream_chunk(h, j):
            xpad = xpads[h]
            st = stage.tile([128, 2048], f32, tag="stage", name="st")
            nc.sync.dma_start(
                out=st[:],
                in_=bass.AP(x_p, h * 128 * HW + j * 2048, [[HW, 128], [1, 2048]]),
            )
            for q in range(4):
                nc.vector.bn_stats(
                    out=sstats[h][:, j * 4 + q, :], in_=st[:, q * 512 : (q + 1) * 512]
                )
            stv = st.rearrange("p (r x) -> p r x", x=128)
            nc.scalar.activation(
                out=xpad[:, 1 + 16 * j : 1 + 16 * (j + 1), 1:129],
                in_=stv,
                func=AF.Copy,
            )
            ra, rb = 1 + 16 * j, 1 + 16 * (j + 1)
            nc.gpsimd.tensor_copy(out=xpad[:, ra:rb, 0:1], in_=xpad[:, ra:rb, 2:3])
            nc.gpsimd.tensor_copy(
                out=xpad[:, ra:rb, 129:130], in_=xpad[:, ra:rb, 127:128]
            )
            if j == 7:
                nc.gpsimd.tensor_copy(out=xpad[:, 0, :], in_=xpad[:, 2, :])
                nc.gpsimd.tensor_copy(out=xpad[:, 129, :], in_=xpad[:, 127, :])

        # ======== fold (pre-stats part): block-diag + W = D^T P ========
        wconv_sb = wq.tile([128, NT * 2, 128], bf16, tag="wconv")
        dbd_sb = wpool.tile([128, NT * 2, 128], bf16, tag="dbd_sb")
        pbd_sb = wpool.tile([128, 2, 128], bf16, tag="pbd_sb")
        f_ps = fpool.tile([128, NT, 128], f32, tag="fps")

        def fold_pre_half(h):
            sl = psrc_sb[:, h, :]
            rep = bass.AP(sl.tensor, sl.offset, [sl.ap[0], [0, 32], [1, 4]])
            nc.vector.tensor_mul(pbd_sb[:, h, :], mask_sb[:], rep)
            for t in range(NT):
                sl = dsrc_sb[:, h, :, t]
                rep = bass.AP(sl.tensor, sl.offset, [sl.ap[0], [0, 32], [NT, 4]])
                nc.vector.tensor_mul(dbd_sb[:, t * 2 + h, :], mask_sb[:], rep)
            for t in range(NT):
                nc.tensor.matmul(
                    out=f_ps[:, t, :],
                    lhsT=dbd_sb[:, t * 2 + h, :],
                    rhs=pbd_sb[:, h, :],
                    start=True,
                    stop=True,
                )

        # ======== fold (post-stats part): a-scaled evac + bias ========
        pwbias2_sb = spool.tile([128, 2], f32, tag="pwbias2")

        def fold_post_half(h):
            nc.vector.bn_aggr(out=mvs[h][:], in_=sstats[h][:])
            a_t = spool.tile([128, 1], f32, tag=f"a{h}", name=f"a{h}")
            nc.scalar.activation(
                out=a_t[:], in_=mvs[h][:, 1:2], func=AF.Sqrt, bias=eps_sb[:]
            )
            nc.vector.reciprocal(out=a_t[:], in_=a_t[:])
            mean_bf = spool.tile([128, 1], bf16, tag=f"mb{h}", name=f"mb{h}")
            nc.vector.tensor_copy(out=mean_bf[:], in_=mvs[h][:, 0:1])
            # wconv[(gg,i), (gg,o)] = a[(gg,i)] * f_ps[(gg,i), (gg,o)] — the
            # in-channel scale lands on the PSUM partition dim, which is the
            # same (gg,i) channel layout as the stats vector.
            for t in range(NT):
                nc.scalar.activation(
                    out=wconv_sb[:, t * 2 + h, :],
                    in_=f_ps[:, t, :],
                    func=AF.Copy,
                    scale=a_t[:],
                )
            for t in range(NT):
                nc.tensor.matmul(
                    out=b_ps_v(h),
                    lhsT=wconv_sb[:, t * 2 + h, :],
                    rhs=mean_bf[:],
                    start=(t == 0),
                    stop=(t == NT - 1),
                )
            nc.vector.tensor_sub(
                pwbias2_sb[:, h : h + 1], pwbias_sb[:, h : h + 1], b_ps_v(h)
            )

        # ======== conv (per half) ========
        def conv_half(h, hook=None):
            xpad = xpads[h]
            for j in range(16):
                ps = cps.tile([128, 1024], f32, tag="conv", name="c_ps")
                for t in range(NT):
                    dy, dx = t // 3, t % 3
                    for hk in range(2):
                        rhs = xpad[
                            :,
                            8 * j + 4 * hk + dy : 8 * j + 4 * hk + dy + 4,
                            dx : dx + 128,
                        ]
                        nc.tensor.matmul(
                            out=ps[:, 512 * hk : 512 * hk + 512],
                            lhsT=wconv_sb[:, t * 2 + h, :],
                            rhs=rhs,
                            start=(t == 0),
                            stop=(t == NT - 1),
                        )
                yt = ypool.tile([128, 1024], bf16, tag="yt", name="yt")
                nc.scalar.activation(
                    out=yt[:],
                    in_=ps[:],
                    func=AF.Identity,
                    bias=pwbias2_sb[:, h : h + 1],
                    scale=1.0,
                )
                nc.gpsimd.dma_start(
                    out=bass.AP(
                        out_p, h * 128 * HW + j * 1024, [[HW, 128], [1, 1024]]
                    ),
                    in_=yt[:],
                )
                if hook is not None and j % 2 == 1:
                    hook(j // 2)

        # ================= emission schedule =================
        # h0: weights (gpsimd queue) + x (sync queue) stream concurrently.
        dwgen_half(0)
        kngen_half(0)
        for j in range(8):
            stream_chunk(0, j)
        fold_pre_half(0)
        fold_post_half(0)

        # h1 weight DMAs ride the sync queue after the x h0 chunks.
        load_weights_half(1, nc.sync)

        def h1_hook(jb):
            stream_chunk(1, jb)
            if jb == 2:
                dwgen_half(1)
            elif jb == 3:
                kngen_half(1)
            elif jb == 4:
                fold_pre_half(1)
            elif jb == 7:
                fold_post_half(1)

        conv_half(0, hook=h1_hook)
        conv_half(1)

    _enforce_wait_budget(nc, mybir)
    return nc


def _enforce_wait_budget(nc, mybir):
    """Hoist excess sync waits into standalone EventSemaphore instructions.

    HW instruction EVENTS hold 1 wait (+1 update); EventSemaphore holds 2.
    Tile occasionally attaches more, which walrus rejects ("Too many sync
    wait commands").  Inserting the extra waits as EventSemaphore ops just
    before the instruction in the same engine stream is semantically
    identical (engine blocks until the semaphore condition holds).
    """
    n_hoisted = 0
    for bname, bassbb in list(nc.bb_map.items()):
        inner = getattr(bassbb, "bb", bassbb)
        insts = inner.instructions
        new_list = []
        for inst in insts:
            cls = type(inst).__name__
            cap = 2 if cls == "InstEventSemaphore" else 1
            si = inst.sync_info
            if si is None:
                new_list.append(inst)
                continue
            waits = list(si.on_wait or [])
            if len(waits) > cap:
                for w in waits[:-cap]:
                    n_hoisted += 1
                    ev = mybir.InstEventSemaphore(
                        name=f"xw-{n_hoisted}-{inst.name}",
                        engine=inst.engine,
                        ins=[],
                        outs=[],
                        sync_info=mybir.SyncInfo(on_wait=[w], on_update=[]),
                    )
                    new_list.append(ev)
                si.on_wait = waits[-cap:]
                inst.sync_info = si
            new_list.append(inst)
        insts[:] = new_list


def _host_prep(style_encoding, predicted, dw_w, dw_b, pw_kn_w, pw_kn_b, pw_b_w, pw_b_b):
    # Device generates dw/pw_kn with channels on PSUM partitions in (gg, m)
    # order, chunked by (h, i) resp. (h, o).  Permute weight output columns
    # accordingly:
    #   perm_d[512h + 128i + 4gg + m] = 16*(32h+gg) + 4m + i
    #   perm_k[512h + 128o + 4gg + m] = 16*(32h+gg) + 4o + m
    hh, ii, gg, mm = np.meshgrid(
        np.arange(2), np.arange(4), np.arange(32), np.arange(4), indexing="ij"
    )
    perm_d = (16 * (32 * hh + gg) + 4 * mm + ii).reshape(-1)
    perm_k = (16 * (32 * hh + gg) + 4 * ii + mm).reshape(-1)  # ii plays 'o' here
    dww_t = np.ascontiguousarray(dw_w.reshape(1024, 2048)[perm_d].T).astype(BF16)
    knw_t = np.ascontiguousarray(pw_kn_w.reshape(1024, SD)[perm_k].T).astype(BF16)
    pbw_t = np.ascontiguousarray(pw_b_w.reshape(C, SD).T).astype(BF16)
    dwb = dw_b[perm_d].reshape(1, 1024).astype(BF16)
    knb = pw_kn_b[perm_k].reshape(1, 1024).astype(BF16)
    pbb = pw_b_b.reshape(1, C).astype(BF16)
    mask = np.kron(np.eye(32, dtype=np.float32), np.ones((4, 4), np.float32)).astype(
        BF16
    )
    in_maps = []
    for n in range(N_CORES):
        s = style_encoding[n]
        cols = np.empty((SD, 2, 2, 3, 3), np.float32)
        for ky in range(2):
            for kx in range(2):
                cols[:, ky, kx] = s[:, ky : ky + 3, kx : kx + 3]
        stycol = np.ascontiguousarray(cols.reshape(2048, NT)).astype(BF16)
        in_maps.append(
            dict(
                x=np.ascontiguousarray(predicted[n].reshape(C, HW)),
                style=np.ascontiguousarray(s.reshape(SD, 16)),
                stycol=stycol,
                dww=dww_t,
                dwb=dwb,
                knw=knw_t,
                knb=knb,
                pbw=pbw_t,
                pbb=pbb,
                mask=mask,
            )
        )
    return in_maps


def get_program():
    global _PROG
    if _PROG is None:
        _PROG = _build_program()
    return _PROG


def run(in_maps, **kw):
    from concourse.bass_utils import run_bass_kernel_spmd

    nc = get_program()
    return run_bass_kernel_spmd(nc, in_maps, core_ids=list(range(N_CORES)), **kw)


def kernel(**inputs):
    in_maps = _host_prep(**{k: np.asarray(v) for k, v in inputs.items()})
    res = run(in_maps)
    out = np.stack(
        [np.asarray(res.results[n]["out"]).reshape(C, H, W) for n in range(N_CORES)]
    )
    return out.astype(np.float32)
